# revision 36
# baseline (speedup 1.0000x reference)
"""LS2T (low-rank signature transform) Trainium2 kernel.

Computes, for X:[B,T,F], kernel:[K,F,U], bias:[K,U] with B=32, T=2048,
F=512, U=128, K=10 (NUM_LEVELS=4):

    M[k] = X @ kernel[k] + bias[k]            (lift, per k)
    Y[0] = sum_t M[0]
    per level m>=2: R = M[k0]; repeat: R = M[k] * exclusive_cumsum_t(R)
    Y[m-1] = sum_t R
    out = stack(Y) : [B, NUM_LEVELS, U]

Strategy (8 NeuronCores, data-parallel over batch, 4 examples/core):
  - Host pre-transposes X to X^T [ex, fchunk, 128f, T] in bf16 so the
    lift matmul contracts f on partitions with no on-device transpose.
    bf16 operands run the PE at 1 row/cycle (fp32r HIGH mode is ~2x
    slower per row) and halve DMA traffic; pipeline rel err ~7e-3,
    well under the 2e-2 gate.
  - Lifts accumulate M[k] as [128u, T] fp32 in PSUM (4 banks,
    double-buffered); chunk-outer/quarter-inner order keeps lhsT
    resident across 4 consecutive matmuls. A few zero-input warmup
    matmuls raise the PE p-state while the first DMAs land.
  - Every M is immediately staged PSUM->SBUF as bf16 on the Scalar
    engine (~2us), so the PE never stalls on PSUM write-after-read.
  - The whole chain runs on the DVE, which is the bottleneck engine
    (~148us/core busy, zero idle): 6 exclusive cumsums per example via
    tensor_tensor_scan (fp32 state, bf16 out, ~4.4us — the scan is
    ALU-latency-bound at 2 cycles/element regardless of dtype), chain
    multiplies via tensor_tensor bf16 (2x_1p, ~1.2us), final reduces
    fused via scalar_tensor_tensor (~2.3us). GpSimd and Scalar stay
    off the DVE's data: concurrent GpSimd tensor_tensor traffic slows
    DVE ops 1.5-2x through SBUF contention (measured), so GpSimd only
    does the one-time memsets.
  - Per example, levels are processed 4 and 3 interleaved (lift order
    6,3,7,4,8,5,9,1,2,0) so both long chains start early; levels 2 and
    1 trail. Scan outputs live in a permanent 4-tile ring whose
    exclusive-scan zero column is memset once.
  - Y columns collect as [128u, 16] via STT/stage accumulators; one PE
    transpose -> [16, 128] -> DMA to DRAM.
"""

import numpy as np
import ml_dtypes

import concourse.bass as bass
from concourse import bacc
import concourse.mybir as mybir
import concourse.tile as tile
from concourse.bass_utils import run_bass_kernel_spmd

# Problem constants (hardcoded per the harness contract)
B, T, F, U = 32, 2048, 512, 128
NUM_LEVELS = 4
K = NUM_LEVELS * (NUM_LEVELS + 1) // 2  # 10
NCORES = 8
EX = B // NCORES  # 4 examples per core
FCH = F // 128  # 4 f-chunks
NQ = T // 512  # 4 PSUM-bank quarters per M tile

F32 = mybir.dt.float32
BF16 = mybir.dt.bfloat16
ALU = mybir.AluOpType
ACTF = mybir.ActivationFunctionType

LIFT_ORDER = [6, 3, 7, 4, 8, 5, 9, 1, 2, 0]


def _emit(nc, tc, xT, kt_d, ident, outd, biasd=None):
    with (
        tc.tile_pool(name="const", bufs=1) as cpool,
        tc.tile_pool(name="xp", bufs=EX) as xpool,
        tc.tile_pool(name="work", bufs=1) as wpool,
    ):
        # --- constants ---
        # kt DMA split per k, first-needed k first, so lift 0 starts early
        kt = cpool.tile([128, K * FCH * U], BF16, tag="kt", name="kt")

        def kdma(k):
            s = slice(k * FCH * U, (k + 1) * FCH * U)
            nc.sync.dma_start(out=kt[:, s], in_=kt_d[:, s])

        kdma(LIFT_ORDER[0])
        xts = []
        xt0 = xpool.tile([128, FCH * T], BF16, tag="xt", name="xt0")
        for c in range(FCH):
            nc.sync.dma_start(out=xt0[:, c * T:(c + 1) * T], in_=xT[0, c])
        xts.append(xt0)
        for k in LIFT_ORDER[1:]:
            kdma(k)
        idt = cpool.tile([128, 128], F32, tag="idt", name="idt")
        nc.sync.dma_start(out=idt, in_=ident[:, :])
        wz = cpool.tile([128, 512], BF16, tag="wz", name="wz")
        nc.gpsimd.memset(wz, 0.0)
        ones16 = cpool.tile([128, T], BF16, tag="ones", name="ones16")
        nc.gpsimd.memset(ones16, 1.0)
        # Permanent scan-output ring: col 0 is the exclusive-scan zero and
        # is memset once — scans only ever write cols 1:T.
        cbs = []
        for i in range(4):
            cb = cpool.tile([128, T + 1], BF16, tag=f"cb{i}", name=f"cb{i}")
            nc.gpsimd.memset(cb[:, 0:1], 0.0)
            cbs.append(cb)
        cb_idx = [0]
        ytile = cpool.tile([128, EX * NUM_LEVELS], F32, tag="y", name="ytile")
        if biasd is not None:
            bias_sb = cpool.tile([128, K], F32, tag="bias", name="bias_sb")
            nc.sync.dma_start(out=bias_sb, in_=biasd[:, :])

        # prefetch the remaining X tiles (DMA streams while PE works)
        for ex in range(1, EX):
            xt = xpool.tile([128, FCH * T], BF16, tag="xt", name=f"xt{ex}")
            for c in range(FCH):
                nc.sync.dma_start(out=xt[:, c * T:(c + 1) * T], in_=xT[ex, c])
            xts.append(xt)

        def kslice(k, c):
            return kt[:, (k * FCH + c) * U:(k * FCH + c + 1) * U]

        # Warm the PE p-state on zeros while the first DMAs land, so the
        # first real lift runs at full clock.
        with tc.tile_pool(name="warm", bufs=1, space="PSUM") as warmpool:
            wp = warmpool.tile([128, 512], F32, tag="wp", name="wp")
            for i in range(6):
                nc.tensor.matmul(
                    wp, lhsT=wz[:, 0:128], rhs=wz, start=True, stop=True
                )

        with tc.tile_pool(name="mp", bufs=2, space="PSUM") as mpool:
            for ex in range(EX):
                xt = xts[ex]

                def lift(k):
                    m = mpool.tile([128, T], F32, tag="m", name=f"m{ex}_{k}")
                    for c in range(FCH):
                        for q in range(NQ):
                            nc.tensor.matmul(
                                m[:, q * 512:(q + 1) * 512],
                                lhsT=kslice(k, c),
                                rhs=xt[:, c * T + q * 512: c * T + (q + 1) * 512],
                                start=(c == 0),
                                stop=(c == FCH - 1),
                            )
                    return m

                def stage(k, m, accum_col=None):
                    """PSUM fp32 -> SBUF bf16 on Scalar; optional Y accum."""
                    ms = wpool.tile(
                        [128, T], BF16, tag="ms", bufs=10, name=f"ms{ex}_{k}"
                    )
                    kw = {}
                    if accum_col is not None:
                        kw["accum_out"] = ytile[:, accum_col:accum_col + 1]
                    if biasd is not None:
                        nc.scalar.activation(
                            out=ms, in_=m, func=ACTF.Identity,
                            bias=bias_sb[:, k:k + 1], **kw,
                        )
                    else:
                        nc.scalar.activation(out=ms, in_=m, func=ACTF.Copy, **kw)
                    return ms

                def scan_excl(src, nm):
                    """Exclusive cumsum along t (DVE, fp32 state, bf16 out)."""
                    cb = cbs[cb_idx[0] % 4]
                    cb_idx[0] += 1
                    nc.vector.tensor_tensor_scan(
                        out=cb[:, 1:T],
                        data0=ones16[:, 0:T - 1],
                        data1=src[:, 0:T - 1],
                        initial=0.0,
                        op0=ALU.mult,
                        op1=ALU.add,
                    )
                    return cb[:, 0:T]

                def vmult(a, c, nm):
                    """bf16 chain multiply on DVE (2x_1p) — keeps the level-4
                    chain DVE-internal (no slow cross-engine hops)."""
                    p = wpool.tile([128, T], BF16, tag="pb", bufs=4, name=f"pb_{nm}")
                    nc.vector.tensor_tensor(out=p, in0=a, in1=c, op=ALU.mult)
                    return p

                def stt_reduce(a, c, lvl, nm):
                    """Fused multiply + t-reduce on DVE (fits in DVE slack)."""
                    sc = wpool.tile([128, T], BF16, tag="st", bufs=2, name=f"st_{nm}")
                    ycol = ex * NUM_LEVELS + lvl
                    nc.vector.scalar_tensor_tensor(
                        out=sc, in0=a, scalar=1.0, in1=c,
                        op0=ALU.mult, op1=ALU.mult,
                        accum_out=ytile[:, ycol:ycol + 1],
                    )

                # Entire chain on DVE in dependency order; levels 4 and 3
                # interleaved so both chains start early. GpSimd stays
                # silent (its Q7 ops collapse under SBUF contention) and
                # Scalar runs only uniform stage copies.
                m6 = lift(6)
                if ex == 0 and biasd is None:
                    # First scan of the kernel: read M6 straight from PSUM
                    # (the DVE is idle anyway, and this starts the chain
                    # earlier). Steady state must stage — a PSUM tile held
                    # for a queued scan would stall the PE.
                    m6s = m6
                else:
                    m6s = stage(6, m6)
                m3 = lift(3)
                m3s = stage(3, m3)
                c6 = scan_excl(m6s, f"{ex}c6")
                m7 = lift(7)
                m7s = stage(7, m7)
                c3 = scan_excl(m3s, f"{ex}c3")
                p7 = vmult(m7s, c6, f"{ex}p7")
                m4 = lift(4)
                m4s = stage(4, m4)
                c7 = scan_excl(p7, f"{ex}c7")
                p4 = vmult(m4s, c3, f"{ex}p4")
                m8 = lift(8)
                m8s = stage(8, m8)
                p8 = vmult(m8s, c7, f"{ex}p8")
                m5 = lift(5)
                m5s = stage(5, m5)
                c8 = scan_excl(p8, f"{ex}c8")
                c4 = scan_excl(p4, f"{ex}c4")
                def reduce_y(a, c, lvl, nm):
                    stt_reduce(a, c, lvl, nm)

                m9 = lift(9)
                m9s = stage(9, m9)
                reduce_y(m9s, c8, 3, f"{ex}y4")
                reduce_y(m5s, c4, 2, f"{ex}y3")
                m1 = lift(1)
                m1s = stage(1, m1)
                m2 = lift(2)
                m2s = stage(2, m2)
                c1 = scan_excl(m1s, f"{ex}c1")
                reduce_y(m2s, c1, 1, f"{ex}y2")
                m0 = lift(0)
                stage(0, m0, accum_col=ex * NUM_LEVELS + 0)  # level 1

        # final transpose of Y: [128u, 16] -> [16, 128u] and store
        with tc.tile_pool(name="yp", bufs=1, space="PSUM") as ypool:
            yps = ypool.tile([EX * NUM_LEVELS, 128], F32, tag="yps", name="yps")
            nc.tensor.matmul(
                yps, lhsT=ytile[:, 0:EX * NUM_LEVELS], rhs=idt,
                start=True, stop=True,
            )
            ysb = wpool.tile([EX * NUM_LEVELS, 128], F32, tag="ysb", name="ysb")
            nc.scalar.activation(out=ysb, in_=yps, func=ACTF.Copy)
            nc.sync.dma_start(out=outd[:, :], in_=ysb)


def build_nc(with_bias):
    nc = bacc.Bacc(trn_type="TRN2", debug=False)
    xT = nc.dram_tensor("xT", [EX, FCH, 128, T], BF16, kind="ExternalInput")
    kt_d = nc.dram_tensor("kt", [128, K * FCH * U], BF16, kind="ExternalInput")
    ident = nc.dram_tensor("ident", [128, 128], F32, kind="ExternalInput")
    biasd = None
    if with_bias:
        biasd = nc.dram_tensor("biasT", [128, K], F32, kind="ExternalInput")
    outd = nc.dram_tensor(
        "out", [EX * NUM_LEVELS, U], F32, kind="ExternalOutput"
    )
    with tile.TileContext(nc) as tc:
        _emit(nc, tc, xT, kt_d, ident, outd, biasd)
    nc.compile()
    return nc


_nc_cache = {}


def _get_nc(with_bias):
    if with_bias not in _nc_cache:
        _nc_cache[with_bias] = build_nc(with_bias)
    return _nc_cache[with_bias]


def make_in_maps(X, kernel, bias, with_bias):
    bf = ml_dtypes.bfloat16
    kt = np.ascontiguousarray(
        kernel.reshape(K, FCH, 128, U).transpose(2, 0, 1, 3)
    ).reshape(128, K * FCH * U).astype(bf)
    ident = np.eye(128, dtype=np.float32)
    Xb = X.astype(bf)  # [B, T, F]
    in_maps = []
    for c in range(NCORES):
        xb = Xb[c * EX:(c + 1) * EX]  # [EX, T, F] bf16
        xT = np.ascontiguousarray(xb.transpose(0, 2, 1)).reshape(EX, FCH, 128, T)
        im = {"xT": xT, "kt": kt, "ident": ident}
        if with_bias:
            im["biasT"] = np.ascontiguousarray(bias.T).astype(np.float32)
        in_maps.append(im)
    return in_maps


def kernel(X, kernel, bias, **run_kwargs):
    X = np.asarray(X, dtype=np.float32)
    kernel = np.asarray(kernel, dtype=np.float32)
    bias = np.asarray(bias, dtype=np.float32)
    with_bias = bool(np.any(bias))
    nc = _get_nc(with_bias)
    in_maps = make_in_maps(X, kernel, bias, with_bias)
    res = run_bass_kernel_spmd(
        nc, in_maps, core_ids=list(range(NCORES)), **run_kwargs
    )
    out = np.concatenate(
        [r["out"].reshape(EX, NUM_LEVELS, U) for r in res.results], axis=0
    )
    if run_kwargs:
        return out, res
    return out
